# revision 2
# baseline (speedup 1.0000x reference)
"""Trainium2 Bass kernel for nn_DAC_structure (sparse dual-attention structure map).

Computes, for inputs q/k of shape (B*CH, L, H, E) = (64, 32, 8, 64):
  s  = softmax((q @ k^T) / sqrt(E))            per (batch-channel, head)
  m  = mean over the CH=8 channel group        -> [b, H, 32, 32]
  out_ps = element-repeat(m_ps, 32, 32)        -> [b, H, 1024, 1024]
  out_pn = tile(m_pn, 32, 32)                  -> [b, H, 1024, 1024]

Sharding: data-parallel over the true batch dim b = 8; core i handles batch i
(channel rows 8i..8i+8). No cross-device comms. Each core writes its own
[8, 1024, 1024] x2 output shard; the host stacks shards along axis 0.
"""

import sys

if "/opt/trn_rl_repo" not in sys.path:
    sys.path.insert(0, "/opt/trn_rl_repo")

from contextlib import ExitStack

import numpy as np

import concourse.bacc as bacc
import concourse.bass as bass
import concourse.mybir as mybir
import concourse.tile as tile
from concourse.masks import make_identity

F32 = mybir.dt.float32

CH = 8   # channels per true batch
L = 32   # patch_num (= seq len of the small attention)
H = 8    # heads
E = 64   # head dim
WIN = 1024
P = 32   # patch size; WIN // P = 32 repeats
N_CORES = 8


def _emit_stream(tc, ctx, pool, psum_t, psum_s, ident, q_dram, k_dram, out_dram, kind):
    """Emit one attention stream (kind='ps' element-repeat or 'pn' block-tile).

    q_dram/k_dram: [CH, L, H, E] DRAM APs; out_dram: [H, WIN, WIN] DRAM AP.
    """
    nc = tc.nc

    # ---- load natural tiles [128 = (c%4)*32 + l, 1024 = (c//4)*512 + h*64 + e]
    nat = {}
    for name, dram in (("q", q_dram), ("k", k_dram)):
        t = pool.tile([128, 1024], F32, tag=f"nat_{kind}_{name}")
        for chalf in range(2):
            src = bass.AP(
                tensor=dram.tensor,
                offset=dram.offset + chalf * 4 * L * H * E,
                ap=[[H * E, 128], [1, H * E]],
            )
            nc.sync.dma_start(out=t[:, chalf * 512 : (chalf + 1) * 512], in_=src)
        nat[name] = t

    # ---- transpose to [128 = (h%2)*64 + e, 1024 = j*128 + (c%4)*32 + l],
    #      j = (c//4)*4 + h//2
    tr = {}
    for name in ("q", "k"):
        t = pool.tile([128, 1024], F32, tag=f"tr_{kind}_{name}")
        for j in range(8):
            pt = psum_t.tile([128, 128], F32, tag="ptrans")
            nc.tensor.transpose(pt, nat[name][:, j * 128 : (j + 1) * 128], ident)
            nc.vector.tensor_copy(t[:, j * 128 : (j + 1) * 128], pt)
        tr[name] = t

    # ---- per h-group g: QK^T matmuls -> softmax -> channel mean -> expand -> DMA out
    for g in range(2):
        s_ps = psum_s.tile([128, 256], F32, tag="spsum")
        for c in range(CH):
            chalf, clo = divmod(c, 4)
            for hh in range(4):
                h = g * 4 + hh
                j = chalf * 4 + h // 2
                col = j * 128 + clo * 32
                prow = (h % 2) * 64
                nc.tensor.matmul(
                    s_ps[hh * 32 : hh * 32 + 32, c * 32 : c * 32 + 32],
                    tr["q"][prow : prow + 64, col : col + 32],
                    tr["k"][prow : prow + 64, col : col + 32],
                    start=True,
                    stop=True,
                    tile_position=(prow, hh * 32),
                )

        # exp(logits / sqrt(E)); no max-subtraction needed (|logits| small)
        ex = pool.tile([128, 256], F32, tag=f"ex_{kind}")
        nc.scalar.activation(ex, s_ps, mybir.ActivationFunctionType.Exp, scale=1.0 / 8.0)

        # per-(c, row) softmax denominators -> reciprocal
        r = pool.tile([128, 8], F32, tag=f"r_{kind}")
        ex_cview = bass.AP(tensor=ex.tensor, offset=ex.offset, ap=[list(ex.ap[0]), [32, 8], [1, 32]])
        nc.vector.tensor_reduce(r, ex_cview, axis=mybir.AxisListType.X, op=mybir.AluOpType.add)
        w = pool.tile([128, 8], F32, tag=f"w_{kind}")
        nc.vector.reciprocal(w, r)

        # weighted channel sum: m = sum_c (ex * (1/CH)) * w_c   -> [128, 32]
        wx = pool.tile([128, 256], F32, tag=f"wx_{kind}")
        ex_scl = bass.AP(tensor=ex.tensor, offset=ex.offset, ap=[list(ex.ap[0]), [1, 32], [32, 8]])
        w_bc = bass.AP(tensor=w.tensor, offset=w.offset, ap=[list(w.ap[0]), [0, 32], [1, 8]])
        wx_out = bass.AP(tensor=wx.tensor, offset=wx.offset, ap=[list(wx.ap[0]), [8, 32], [1, 8]])
        nc.vector.scalar_tensor_tensor(
            out=wx_out, in0=ex_scl, scalar=1.0 / CH, in1=w_bc,
            op0=mybir.AluOpType.mult, op1=mybir.AluOpType.mult,
        )
        m = pool.tile([128, 32], F32, tag=f"m_{kind}")
        wx_in = bass.AP(tensor=wx.tensor, offset=wx.offset, ap=[list(wx.ap[0]), [8, 32], [1, 8]])
        nc.vector.tensor_reduce(m, wx_in, axis=mybir.AxisListType.X, op=mybir.AluOpType.add)

        # expand rows to 1024 wide and DMA the [4, 1024, 1024] h-group out
        exp_t = pool.tile([128, 1024], F32, tag=f"expand_{kind}")
        if kind == "ps":
            # element-repeat within row: E[p, 32*s + t] = m[p, s]
            src = bass.AP(tensor=m.tensor, offset=m.offset, ap=[list(m.ap[0]), [1, 32], [0, 32]])
            nc.vector.tensor_copy(exp_t, src)
            # one 16MB DMA: partition p = (h%4)*32 + rowblock -> 32 output rows
            src2 = bass.AP(tensor=exp_t.tensor, offset=exp_t.offset,
                           ap=[list(exp_t.ap[0]), [0, 32], [1, 1024]])
            dst = bass.AP(tensor=out_dram.tensor,
                          offset=out_dram.offset + g * 4 * WIN * WIN,
                          ap=[[32 * WIN, 128], [WIN, 32], [1, 1024]])
            nc.sync.dma_start(out=dst, in_=src2)
        else:
            # tile row: T[p, 32*u + s] = m[p, s]
            src = bass.AP(tensor=m.tensor, offset=m.offset, ap=[list(m.ap[0]), [0, 32], [1, 32]])
            nc.vector.tensor_copy(exp_t, src)
            # per h: partition t -> output rows {u*32 + t}
            for hh in range(4):
                h = g * 4 + hh
                sl = exp_t[hh * 32 : hh * 32 + 32, :]
                src2 = bass.AP(tensor=sl.tensor, offset=sl.offset,
                               ap=[list(sl.ap[0]), [0, 32], [1, 1024]])
                dst = bass.AP(tensor=out_dram.tensor,
                              offset=out_dram.offset + h * WIN * WIN,
                              ap=[[WIN, 32], [32 * WIN, 32], [1, 1024]])
                nc.sync.dma_start(out=dst, in_=src2)


def build_program():
    """Build and compile the per-core Bass program. Returns the Bacc object."""
    nc = bacc.Bacc(
        "TRN2",
        target_bir_lowering=False,
        debug=False,
        enable_asserts=False,
        num_devices=N_CORES,
    )
    ins = {}
    for name in ("qps", "qpn", "kps", "kpn"):
        ins[name] = nc.dram_tensor(name, [CH, L, H, E], F32, kind="ExternalInput").ap()
    out_ps = nc.dram_tensor("out_ps", [H, WIN, WIN], F32, kind="ExternalOutput").ap()
    out_pn = nc.dram_tensor("out_pn", [H, WIN, WIN], F32, kind="ExternalOutput").ap()

    with tile.TileContext(nc) as tc:
        with ExitStack() as ctx:
            pool = ctx.enter_context(tc.tile_pool(name="sbuf", bufs=1))
            psum_t = ctx.enter_context(tc.tile_pool(name="ptrans", bufs=3, space="PSUM"))
            psum_s = ctx.enter_context(tc.tile_pool(name="spsum", bufs=2, space="PSUM"))
            ident = pool.tile([128, 128], F32, tag="ident")
            make_identity(nc, ident)
            _emit_stream(tc, ctx, pool, psum_t, psum_s, ident,
                         ins["qps"], ins["kps"], out_ps, "ps")
            _emit_stream(tc, ctx, pool, psum_t, psum_s, ident,
                         ins["qpn"], ins["kpn"], out_pn, "pn")
    nc.compile()
    return nc


_NC_CACHE = None


def _get_nc():
    global _NC_CACHE
    if _NC_CACHE is None:
        _NC_CACHE = build_program()
    return _NC_CACHE


def run_sharded(queries_patch_size, queries_patch_num, keys_patch_size, keys_patch_num,
                trace=False):
    """Run the SPMD kernel on 8 cores; returns (full_ps, full_pn[, results])."""
    from concourse.bass_utils import run_bass_kernel_spmd

    nc = _get_nc()
    qps = np.ascontiguousarray(np.asarray(queries_patch_size, dtype=np.float32))
    qpn = np.ascontiguousarray(np.asarray(queries_patch_num, dtype=np.float32))
    kps = np.ascontiguousarray(np.asarray(keys_patch_size, dtype=np.float32))
    kpn = np.ascontiguousarray(np.asarray(keys_patch_num, dtype=np.float32))

    in_maps = []
    for b in range(N_CORES):
        sl = slice(b * CH, (b + 1) * CH)
        in_maps.append({
            "qps": qps[sl], "qpn": qpn[sl], "kps": kps[sl], "kpn": kpn[sl],
        })
    res = run_bass_kernel_spmd(nc, in_maps, core_ids=list(range(N_CORES)), trace=trace)
    full_ps = np.stack([res.results[b]["out_ps"] for b in range(N_CORES)], axis=0)
    full_pn = np.stack([res.results[b]["out_pn"] for b in range(N_CORES)], axis=0)
    if trace:
        return full_ps, full_pn, res
    return full_ps, full_pn


def kernel(queries_patch_size, queries_patch_num, keys_patch_size, keys_patch_num,
           values=None, patch_index=0, attn_mask=None):
    """Full-input entry point: takes the unsharded inputs, returns full outputs."""
    full_ps, full_pn = run_sharded(
        queries_patch_size, queries_patch_num, keys_patch_size, keys_patch_num
    )
    return full_ps, full_pn


# revision 4
# speedup vs baseline: 1.2935x; 1.2935x over previous
"""Trainium2 Bass kernel for nn_DAC_structure (sparse dual-attention structure map).

For inputs q/k of shape (B*CH, L, H, E) = (64, 32, 8, 64):
  s  = softmax((q @ k^T) / sqrt(E))            per (batch-channel, head)
  m  = mean over the CH=8 channel group        -> [b, H, 32, 32]
  out_ps = element-repeat(m_ps, 32, 32)        -> [b, H, 1024, 1024]
  out_pn = tile(m_pn, 32, 32)                  -> [b, H, 1024, 1024]

Sharding: data-parallel over the true batch dim b = 8; core i handles batch i
(channel rows 8i..8i+8). No cross-device comms. Each core writes its own
[8, 1024, 1024] x2 output shard; the host stacks shards along axis 0.

The kernel is HBM-write-bound (64 MB out per core). Both expansions are
written with fully sequential HBM address streams:
  - out_ps: one 16 MB DMA per 4-head group, source rows re-read 32x via a
    stride-0 middle AP dim (partition-major walk is already sequential).
  - out_pn: the 32-row tile block is partition-replicated 4x into a
    [128, 1024] SBUF tile (= 128 consecutive output rows), then written with
    eight flat 512 KB DMAs per head. (A strided t-outer walk measures
    ~205 GB/s on HBM vs ~360-410 GB/s for these sequential writes.)
"""

import sys

if "/opt/trn_rl_repo" not in sys.path:
    sys.path.insert(0, "/opt/trn_rl_repo")

from contextlib import ExitStack

import numpy as np

import concourse.bacc as bacc
import concourse.bass as bass
import concourse.mybir as mybir
import concourse.tile as tile
from concourse.masks import make_identity

F32 = mybir.dt.float32

CH = 8   # channels per true batch
L = 32   # patch_num (seq len of the small attention)
H = 8    # heads
E = 64   # head dim
WIN = 1024
N_CORES = 8


def _load_inputs(nc, pool, ins, kind):
    """One DMA per tensor into [128 = (c%4)*32 + l, 1024 = (c//4)*512 + h*64 + e]."""
    eng = nc.sync if kind == "ps" else nc.scalar
    nat = {}
    for name in ("q", "k"):
        dram = ins[name]
        t = pool.tile([128, 1024], F32, tag=f"nat_{kind}_{name}")
        pitch = t.ap[0][0]
        src = bass.AP(tensor=dram.tensor, offset=dram.offset,
                      ap=[[H * E, 128], [4 * L * H * E, 2], [1, H * E]])
        dst = bass.AP(tensor=t.tensor, offset=t.offset,
                      ap=[[pitch, 128], [512, 2], [1, 512]])
        eng.dma_start(out=dst, in_=src)
        nat[name] = t
    return nat


def _transpose_group(nc, pool, psum_t, ident, nat, tr, g, kind):
    """PE-transpose the 128-col chunks needed by h-group g into tr tiles."""
    js = (0, 1, 4, 5) if g == 0 else (2, 3, 6, 7)
    for name in ("q", "k"):
        for j in js:
            pt = psum_t.tile([128, 128], F32, tag="ptrans")
            nc.tensor.transpose(pt, nat[name][:, j * 128 : (j + 1) * 128], ident)
            nc.vector.tensor_copy(tr[name][:, j * 128 : (j + 1) * 128], pt)


def _group_mean_softmax(nc, pool, psum_s, tr, g, kind):
    """QK^T matmuls + softmax + channel mean for h-group g -> M [128, 32]."""
    s_ps = psum_s.tile([128, 256], F32, tag="spsum")
    for c in range(CH):
        chalf, clo = divmod(c, 4)
        for hh in range(4):
            h = g * 4 + hh
            col = (chalf * 4 + h // 2) * 128 + clo * 32
            prow = (h % 2) * 64
            nc.tensor.matmul(
                s_ps[hh * 32 : hh * 32 + 32, c * 32 : c * 32 + 32],
                tr["q"][prow : prow + 64, col : col + 32],
                tr["k"][prow : prow + 64, col : col + 32],
                start=True, stop=True,
                tile_position=(prow, hh * 32),
            )
    ex = pool.tile([128, 256], F32, tag=f"ex_{kind}")
    nc.scalar.activation(ex, s_ps, mybir.ActivationFunctionType.Exp, scale=1.0 / 8.0)
    r = pool.tile([128, 8], F32, tag=f"r_{kind}")
    ex_cview = bass.AP(tensor=ex.tensor, offset=ex.offset,
                       ap=[list(ex.ap[0]), [32, 8], [1, 32]])
    nc.vector.tensor_reduce(r, ex_cview, axis=mybir.AxisListType.X, op=mybir.AluOpType.add)
    w = pool.tile([128, 8], F32, tag=f"w_{kind}")
    nc.vector.reciprocal(w, r)
    wx = pool.tile([128, 256], F32, tag=f"wx_{kind}")
    ex_scl = bass.AP(tensor=ex.tensor, offset=ex.offset,
                     ap=[list(ex.ap[0]), [1, 32], [32, 8]])
    w_bc = bass.AP(tensor=w.tensor, offset=w.offset,
                   ap=[list(w.ap[0]), [0, 32], [1, 8]])
    wx_out = bass.AP(tensor=wx.tensor, offset=wx.offset,
                     ap=[list(wx.ap[0]), [8, 32], [1, 8]])
    nc.vector.scalar_tensor_tensor(out=wx_out, in0=ex_scl, scalar=1.0 / CH, in1=w_bc,
                                   op0=mybir.AluOpType.mult, op1=mybir.AluOpType.mult)
    m = pool.tile([128, 32], F32, tag=f"m_{kind}")
    wx_in = bass.AP(tensor=wx.tensor, offset=wx.offset,
                    ap=[list(wx.ap[0]), [8, 32], [1, 8]])
    nc.vector.tensor_reduce(m, wx_in, axis=mybir.AxisListType.X, op=mybir.AluOpType.add)
    return m


def _emit_ps_group(nc, pool, m, out_dram, g):
    """Element-repeat expansion + one sequential 16MB DMA for h-group g."""
    exp_t = pool.tile([128, 1024], F32, tag="expand_ps")
    src = bass.AP(tensor=m.tensor, offset=m.offset,
                  ap=[list(m.ap[0]), [1, 32], [0, 32]])
    nc.vector.tensor_copy(exp_t, src)
    pitch = exp_t.ap[0][0]
    src2 = bass.AP(tensor=exp_t.tensor, offset=exp_t.offset,
                   ap=[[pitch, 128], [0, 32], [1, 1024]])
    dst = bass.AP(tensor=out_dram.tensor,
                  offset=out_dram.offset + g * 4 * WIN * WIN,
                  ap=[[32 * WIN, 128], [WIN, 32], [1, 1024]])
    nc.sync.dma_start(out=dst, in_=src2)


def _emit_pn_group(nc, pool, m, out_dram, g):
    """Tile expansion: replicate to 128 consecutive rows, 8x 512KB flat DMAs/head."""
    exp_t = pool.tile([128, 1024], F32, tag="expand_pn")
    src = bass.AP(tensor=m.tensor, offset=m.offset,
                  ap=[list(m.ap[0]), [0, 32], [1, 32]])
    nc.vector.tensor_copy(exp_t, src)
    for hh in range(4):
        h = g * 4 + hh
        rep = pool.tile([128, 1024], F32, tag=f"rep_pn_{h % 2}")
        for u_lo in range(4):
            nc.scalar.dma_start(out=rep[u_lo * 32 : (u_lo + 1) * 32, :],
                                in_=exp_t[hh * 32 : (hh + 1) * 32, :])
        for u_hi in range(8):
            dst = bass.AP(tensor=out_dram.tensor,
                          offset=out_dram.offset + (h * WIN + u_hi * 128) * WIN,
                          ap=[[WIN, 128], [1, 1024]])
            nc.scalar.dma_start(out=dst, in_=rep[:, :])


def build_program():
    """Build and compile the per-core Bass program. Returns the Bacc object."""
    nc = bacc.Bacc(
        "TRN2",
        target_bir_lowering=False,
        debug=False,
        enable_asserts=False,
        num_devices=N_CORES,
    )
    ins = {}
    for name in ("qps", "qpn", "kps", "kpn"):
        ins[name] = nc.dram_tensor(name, [CH, L, H, E], F32, kind="ExternalInput").ap()
    out_ps = nc.dram_tensor("out_ps", [H, WIN, WIN], F32, kind="ExternalOutput").ap()
    out_pn = nc.dram_tensor("out_pn", [H, WIN, WIN], F32, kind="ExternalOutput").ap()

    with tile.TileContext(nc) as tc:
        with ExitStack() as ctx:
            pool = ctx.enter_context(tc.tile_pool(name="sbuf", bufs=1))
            rep_pool = ctx.enter_context(tc.tile_pool(name="reps", bufs=2))
            psum_t = ctx.enter_context(tc.tile_pool(name="ptrans", bufs=3, space="PSUM"))
            psum_s = ctx.enter_context(tc.tile_pool(name="spsum", bufs=2, space="PSUM"))
            ident = pool.tile([128, 128], F32, tag="ident")
            make_identity(nc, ident)

            nat_ps = _load_inputs(nc, pool, {"q": ins["qps"], "k": ins["kps"]}, "ps")
            nat_pn = _load_inputs(nc, pool, {"q": ins["qpn"], "k": ins["kpn"]}, "pn")
            tr_ps = {n: pool.tile([128, 1024], F32, tag=f"tr_ps_{n}",
                                  name=f"tr_ps_{n}") for n in ("q", "k")}
            tr_pn = {n: pool.tile([128, 1024], F32, tag=f"tr_pn_{n}",
                                  name=f"tr_pn_{n}") for n in ("q", "k")}

            # ps g0 first: gets the first big sequential DMA going ASAP
            _transpose_group(nc, pool, psum_t, ident, nat_ps, tr_ps, 0, "ps")
            m = _group_mean_softmax(nc, pool, psum_s, tr_ps, 0, "ps")
            _emit_ps_group(nc, pool, m, out_ps, 0)

            _transpose_group(nc, pool, psum_t, ident, nat_pn, tr_pn, 0, "pn")
            m = _group_mean_softmax(nc, pool, psum_s, tr_pn, 0, "pn")
            _emit_pn_group(nc, rep_pool, m, out_pn, 0)

            _transpose_group(nc, pool, psum_t, ident, nat_ps, tr_ps, 1, "ps")
            m = _group_mean_softmax(nc, pool, psum_s, tr_ps, 1, "ps")
            _emit_ps_group(nc, pool, m, out_ps, 1)

            _transpose_group(nc, pool, psum_t, ident, nat_pn, tr_pn, 1, "pn")
            m = _group_mean_softmax(nc, pool, psum_s, tr_pn, 1, "pn")
            _emit_pn_group(nc, rep_pool, m, out_pn, 1)
    nc.compile()
    return nc


_NC_CACHE = None


def _get_nc():
    global _NC_CACHE
    if _NC_CACHE is None:
        _NC_CACHE = build_program()
    return _NC_CACHE


def run_sharded(queries_patch_size, queries_patch_num, keys_patch_size, keys_patch_num,
                trace=False, tmpdir=None):
    """Run the SPMD kernel on 8 cores; returns (full_ps, full_pn[, results])."""
    from concourse.bass_utils import run_bass_kernel_spmd

    nc = _get_nc()
    qps = np.ascontiguousarray(np.asarray(queries_patch_size, dtype=np.float32))
    qpn = np.ascontiguousarray(np.asarray(queries_patch_num, dtype=np.float32))
    kps = np.ascontiguousarray(np.asarray(keys_patch_size, dtype=np.float32))
    kpn = np.ascontiguousarray(np.asarray(keys_patch_num, dtype=np.float32))

    in_maps = []
    for b in range(N_CORES):
        sl = slice(b * CH, (b + 1) * CH)
        in_maps.append({
            "qps": qps[sl], "qpn": qpn[sl], "kps": kps[sl], "kpn": kpn[sl],
        })
    res = run_bass_kernel_spmd(nc, in_maps, core_ids=list(range(N_CORES)), trace=trace,
                               tmpdir=tmpdir)
    full_ps = np.stack([res.results[b]["out_ps"] for b in range(N_CORES)], axis=0)
    full_pn = np.stack([res.results[b]["out_pn"] for b in range(N_CORES)], axis=0)
    if trace:
        return full_ps, full_pn, res
    return full_ps, full_pn


def kernel(queries_patch_size, queries_patch_num, keys_patch_size, keys_patch_num,
           values=None, patch_index=0, attn_mask=None):
    """Full-input entry point: takes the unsharded inputs, returns full outputs."""
    full_ps, full_pn = run_sharded(
        queries_patch_size, queries_patch_num, keys_patch_size, keys_patch_num
    )
    return full_ps, full_pn
